# revision 43
# baseline (speedup 1.0000x reference)
# NonLocalBlock Trainium2 Bass kernel.
#
# Reference computation (per batch b):
#   theta = theta_w @ X + theta_b          [IC, N]   (X = x[b] as [C, N])
#   phi   = phi_w   @ X + phi_b            [IC, N]
#   g     = g_w     @ X + g_b              [IC, N]
#   attn  = softmax_j(theta^T phi)         [N, N]
#   att   = g @ attn^T                     [IC, N]
#   y     = BN(w_w @ att + w_b) + x
#
# Math folds used on device (validated vs reference):
#   - phi bias drops out of softmax entirely (adds an i-only constant).
#   - g bias folds into the final bias because attn rows sum to 1.
#   - BN is affine: fold into w_eff = inv*w_w and b_final.
#   - scores bounded (|s| < 50) so exp() needs no max-subtraction.
#
# Sharding: 8 cores = 4 batches x 2 row-halves. Each core receives x[b]
# with its own half's columns swapped to the front, so every core runs an
# identical program (pure SPMD): it projects theta for columns 0..2047
# ("own" rows i) and phi/g for all 4096 columns (keys/values j), computes
# 2048x4096 attention flash-style, and emits y for its own 2048 columns.
#
# Precision plan (validated vs reference in numpy, absmax-rel ~3.0e-3):
#   - x and the three projection weights ship as fp16 (halves the input
#     DMA, which floor-limits the startup) — fp16's 10 mantissa bits keep
#     the softmax scores accurate where bf16 would cost ~1.6e-2.
#   - theta/phi live in SBUF as fp32r; scores accumulate fp32 in PSUM.
#   - value path (exp output, gT, attn, w_eff) is bf16: attention-weighted
#     averaging damps value-path quantization.
#   - output ships fp16 (host upconverts); residual adds use fp16 x.
#
# Denominator: softmax denominators need a cross-partition sum, which only
# the PE (ones-matmul) can do cheaply.  exp groups are pairwise-summed on
# DVE (bf16, 2x mode) twice, so only 8+2 ones-matmuls per i-block reach
# the PE; the final pair skips the presum and feeds the PE directly so the
# block-flush critical path doesn't wait on the DVE queue.
#
# Schedule notes (from HW traces):
#   - a short burst of 128-col bf16 warmup matmuls at t=0 starts the PE
#     HAM clock ramp; a dummy exp preloads the ACT exp table.
#   - x streams over the sync (C-rows 0:128) and gpsimd (128:256) DMA
#     rings; ALL weights go on the scalar ring so no weight transfer ever
#     delays an x slice.  gT is produced directly by x-chunk-stationary
#     matmuls (no PE transposes, no second copy).
#   - per 512-col slice t: projections, then attention groups (2t, 2t+1)
#     of block 0 — the exp stream starts ~10us in and paces the kernel.
#   - exp consumption (AV matmuls + presums) runs DEFER groups late; block
#     tails (reciprocal/normalize/W/store) interleave with the next
#     block's first groups so no engine FIFO ever stalls on them.

from contextlib import ExitStack

import numpy as np

import concourse.bass as bass
import concourse.tile as tile
from concourse import bacc, mybir
from concourse.bass_utils import run_bass_kernel_spmd

F32 = mybir.dt.float32
F32R = mybir.dt.float32r
F16 = mybir.dt.float16
BF16 = mybir.dt.bfloat16
AF = mybir.ActivationFunctionType
ALU = mybir.AluOpType

B, C, IC = 4, 256, 128
H = W = 64
N = H * W            # 4096
HALF = N // 2        # 2048 rows of attention per core
P = 128
NCORES = 8
NBLK = HALF // 512   # 4 i-blocks of 512
NCH = N // P         # 32 j-chunks of 128
NGRP = NCH // 2      # 16 groups of 2 chunks per i-block
DEFER = 4            # consume exp output this many groups late
NWARM = 48           # HAM warmup matmuls at t=0 (128-col bf16, ~5.1us)
# Groups whose exp runs on DVE via bf16 Schraudolph (bit-trick exp: bf16
# bits of e^s are ~ s*128/ln2 + 127*128 - C) instead of the saturated ACT
# engine.  3 of 16 groups rebalances ACT 18.4->14.9us/block while DVE
# stays under the PE pace.  Costs ~5.5e-3 absmax-rel (validated, C=5).
SCHRAUD_GRPS = (4, 12)
SCH_A = 128.0 / float(np.log(2.0))
SCH_B = 127.0 * 128.0 - 5.0
BN_EPS = 1e-5


def _r(ap):
    return ap.bitcast(F32R)


def _emit_consume(nc, pools, blk, grp, only=None):
    """AV matmuls + denominator work for group `grp`.

    Denominator: groups 0..13 run a bf16 presum ladder on DVE (pairs of
    chunks, then pairs of groups) feeding one ones-matmul per group-pair;
    groups 14/15 feed the ones-matmul directly so the block flush never
    waits on the DVE queue."""
    att_ps = pools["att_ps"][blk]
    den_ps = pools["den_ps"][blk]
    gT_sb, onesP_sb = pools["gT_sb"], pools["onesP_sb"]
    p_pool = pools["pre"]
    ex_sb = pools["ex_sbs"][(blk, grp)]
    if only in (None, "den"):
        # the direct (no-presum) path only serves the LAST block's final
        # pair, where it keeps the end-of-kernel chain off the DVE queue;
        # elsewhere the presum ladder saves PE matmuls.
        if grp < NGRP - 2 or blk < NBLK - 1:
            p1 = p_pool.tile([P, 512], BF16, name=f"p1_{blk}_{grp}",
                             tag="p1", bufs=3)
            nc.vector.tensor_add(p1[:], ex_sb[:, 0:512], ex_sb[:, 512:1024])
            pools["p1_sbs"][(blk, grp)] = p1
            if grp % 2 == 1:
                pa = pools["p1_sbs"].pop((blk, grp - 1))
                p2 = p_pool.tile([P, 512], BF16, name=f"p2_{blk}_{grp // 2}",
                                 tag="p2", bufs=3)
                nc.vector.tensor_add(p2[:], pa[:], p1[:])
                nc.tensor.matmul(den_ps[:], onesP_sb[:], p2[:],
                                 start=grp == 1, stop=grp == NGRP - 1)
        else:
            for c in range(2):
                jc = grp * 2 + c
                nc.tensor.matmul(
                    den_ps[:], onesP_sb[:],
                    ex_sb[:, c * 512:(c + 1) * 512],
                    start=False, stop=jc == NCH - 1)
    if only in (None, "av"):
        for c in range(2):
            jc = grp * 2 + c
            nc.tensor.matmul(
                att_ps[:], gT_sb[:, jc * P:(jc + 1) * P],
                ex_sb[:, c * 512:(c + 1) * 512],
                start=jc == 0, stop=jc == NCH - 1)


def _emit_group(nc, pools, blk, grp):
    """Scores + exp for one [128,1024] group, consuming DEFER groups late."""
    ps_pool, ex_pool = pools["ps"], pools["ex"]
    theta_sb, phi_sb = pools["theta_sb"], pools["phi_sb"]
    isl = slice(blk * 512, (blk + 1) * 512)
    if grp == 0:
        pools["att_ps"][blk] = ps_pool.tile(
            [P, 512], F32, name=f"att_ps{blk}", tag="att", bufs=1)
        pools["den_ps"][blk] = ps_pool.tile(
            [P, 512], F32, name=f"den_ps{blk}", tag="den", bufs=1)
    sc_ps = ps_pool.tile([P, 1024], F32, name=f"sc{blk}_{grp}", tag="sc",
                         bufs=2)
    for c in range(2):
        jc = grp * 2 + c
        nc.tensor.matmul(
            sc_ps[:, c * 512:(c + 1) * 512],
            phi_sb[:, jc * P:(jc + 1) * P],
            theta_sb[:, isl],
            start=True, stop=True)
    ex_sb = ex_pool.tile([P, 1024], BF16, name=f"ex{blk}_{grp}", tag="ex")
    pools["ex_sbs"][(blk, grp)] = ex_sb
    if grp in SCHRAUD_GRPS:
        nc.vector.tensor_scalar(
            out=ex_sb[:].bitcast(mybir.dt.int16), in0=sc_ps[:],
            scalar1=SCH_A, scalar2=SCH_B, op0=ALU.mult, op1=ALU.add)
    else:
        nc.scalar.activation(ex_sb[:], sc_ps[:], AF.Exp)
    if grp >= DEFER:
        _emit_consume(nc, pools, blk, grp - DEFER)


def _emit_recip(nc, pools, blk):
    """1/den — emitted right after the block's den flush so it frees the
    den PSUM slot early and leads the DVE FIFO."""
    rec_pool = pools["rec"]
    den_ps = pools["den_ps"][blk]
    recb = rec_pool.tile([P, 512], F32, name=f"recb{blk}", tag="recb")
    nc.vector.reciprocal_approx_fast(out=recb[:], in_=den_ps[:])
    pools["recbs"][blk] = recb


def _emit_attnmul(nc, pools, blk):
    """attn = att * (1/den) — must follow the block's last AV matmul."""
    rec_pool = pools["rec"]
    att_ps = pools["att_ps"][blk]
    recb = pools["recbs"].pop(blk)
    attn_sb = rec_pool.tile([P, 512], BF16, name=f"attn{blk}", tag="attn")
    nc.vector.tensor_mul(attn_sb[:], att_ps[:], recb[:])
    pools["attn_sbs"][blk] = attn_sb


def _emit_tail_pe(nc, pools, blk, yout, last=False):
    """W projection, bias+residual, fp16 store for block `blk`."""
    ps_pool, rec_pool = pools["ps"], pools["rec"]
    wef_sb, bfin_sb = pools["wef_sb"], pools["bfin_sb"]
    x_sb = pools["x_sb"]
    attn_sb = pools["attn_sbs"].pop(blk)
    isl = slice(blk * 512, (blk + 1) * 512)
    for k in range(2):
        y_ps = ps_pool.tile([P, 512], F32, name=f"y{blk}_{k}", tag="pp",
                            bufs=2)
        nc.tensor.matmul(
            y_ps[:], wef_sb[:, k * P:(k + 1) * P], attn_sb[:],
            start=True, stop=True)
        yo = rec_pool.tile([P, 512], F16, name=f"yo{blk}_{k}", tag="yo")
        # yo = (y + b_final) + x  in one DVE op
        nc.vector.scalar_tensor_tensor(
            out=yo[:], in0=y_ps[:], scalar=bfin_sb[:, k:k + 1],
            in1=x_sb[k][:, isl], op0=ALU.add, op1=ALU.add)
        if last:
            eng = nc.sync if k == 0 else nc.scalar
        else:
            eng = nc.sync if k == 0 else nc.gpsimd
        eng.dma_start(out=yout[k * P:(k + 1) * P, isl], in_=yo[:])


def _kernel_body(ctx, tc, ins, yout):
    nc = tc.nc
    xin, wef, tb, bfin = ins["xin"], ins["wef"], ins["tb"], ins["bfin"]

    consts = ctx.enter_context(tc.tile_pool(name="consts", bufs=1))
    big = ctx.enter_context(tc.tile_pool(name="big", bufs=1))

    # ---- dummy tiles for HAM warmup (bf16 128-col matmuls) ----
    dum_f = consts.tile([P, P], F32, name="dum_f")
    nc.vector.memset(dum_f[:], 1.0)
    dum_b = consts.tile([P, P], BF16, name="dum_b")
    nc.vector.tensor_copy(dum_b[:], dum_f[:])

    # ---- x load: k=0 C-half on the sync ring, k=1 on the gpsimd ring;
    # all weights on the scalar ring (idle until the exps start).
    x_sb = [big.tile([P, N], F16, name=f"x_sb{k}") for k in range(2)]

    def xdma(t, k, eng):
        tsl = slice(t * 512, (t + 1) * 512)
        eng.dma_start(out=x_sb[k][:, tsl],
                      in_=xin[k * P:(k + 1) * P, tsl])

    wcat_sb = consts.tile([P, 2 * 384], F16, name="wcat_sb")
    tb_sb = consts.tile([P, 1], F32, name="tb_sb")
    wef_f = consts.tile([P, C], F32, name="wef_f")
    bfin_sb = consts.tile([P, 2], F32, name="bfin_sb")
    wcat = ins["wcat"]

    def xchunk(c0, c1, k, eng):
        eng.dma_start(out=x_sb[k][:, c0:c1],
                      in_=xin[k * P:(k + 1) * P, c0:c1])

    # sync and scalar are hardware-dynamic DMA queues; the gpsimd queue is
    # software-dynamic (~2x slower), so it only gets the tail k=1 chunk
    # (~8us of slack).  A queue's first couple of descriptors cost ~2us
    # each before transfers pipeline, so thw|phw|gw ship as ONE
    # concatenated [C, 384] array (wcat) in two descriptors at the head of
    # the fast sync queue, with x right behind in first-use order.
    nc.sync.dma_start(out=wcat_sb[:].rearrange("p (k m) -> p k m", k=2),
                      in_=wcat.rearrange("(k p) m -> p k m", p=P))
    nc.scalar.dma_start(out=tb_sb[:], in_=tb[:, None])
    exdum = consts.tile([P, 1], F32, name="exdum")
    nc.scalar.activation(exdum[:], dum_f[:, 0:1], AF.Exp)  # load exp table
    xchunk(0, 512, 0, nc.sync)
    xchunk(0, 512, 1, nc.sync)
    xchunk(512, 1024, 0, nc.sync)
    xchunk(512, 1024, 1, nc.sync)
    xchunk(1024, 2048, 0, nc.sync)
    xchunk(2048, 3072, 0, nc.sync)
    xchunk(3072, 4096, 0, nc.sync)
    xchunk(1024, 2048, 1, nc.scalar)
    xchunk(2048, 4096, 1, nc.gpsimd)
    nc.scalar.dma_start(out=wef_f[:], in_=wef[:, :])
    nc.scalar.dma_start(out=bfin_sb[:],
                        in_=bfin.rearrange("(k p) -> p k", p=P))

    def thw_sl(k):
        return wcat_sb[:, k * 384:k * 384 + P]

    def phw_sl(k):
        return wcat_sb[:, k * 384 + P:k * 384 + 2 * P]

    def gw_sl(k):
        return wcat_sb[:, k * 384 + 2 * P:k * 384 + 3 * P]

    wef_sb = consts.tile([P, C], BF16, name="wef_sb")
    onesP_sb = consts.tile([P, P], BF16, name="onesP_sb")
    nc.vector.memset(onesP_sb[:], 1.0)

    theta_sb = big.tile([P, HALF], F32R, name="theta_sb")
    phi_sb = big.tile([P, N], F32R, name="phi_sb")
    gT_sb = big.tile([P, N], BF16, name="gT_sb")

    # ---- single PSUM pool, tagged slots (8 banks total):
    #   sc 2x[128,1024]=4, att 1, den 1, pp 2x[128,512]=2 (proj/gT/y)
    ps_pool = ctx.enter_context(tc.tile_pool(name="ps", bufs=1, space="PSUM"))
    pools = {
        "ps": ps_pool,
        "ex": ctx.enter_context(tc.tile_pool(name="ex", bufs=5 + DEFER)),
        "pre": ctx.enter_context(tc.tile_pool(name="pre", bufs=1)),
        "rec": ctx.enter_context(tc.tile_pool(name="rec", bufs=2)),
        "theta_sb": theta_sb, "phi_sb": phi_sb, "gT_sb": gT_sb,
        "onesP_sb": onesP_sb, "wef_sb": wef_sb, "bfin_sb": bfin_sb,
        "x_sb": x_sb,
        "att_ps": {}, "den_ps": {}, "ex_sbs": {},
        "p1_sbs": {}, "attn_sbs": {}, "recbs": {},
    }

    # ---- phase 1: per-slice projections + block-0 groups as x lands ----
    dum_ps = ps_pool.tile([P, 512], F32, name="dum_ps", tag="pp", bufs=2)
    for i in range(NWARM):
        nc.tensor.matmul(dum_ps[:, 0:P], dum_b[:], dum_b[:],
                         start=True, stop=True)

    def proj(t):
        tsl = slice(t * 512, (t + 1) * 512)
        if t < NBLK:
            ps = ps_pool.tile([P, 512], F32, name=f"th_ps{t}", tag="pp",
                              bufs=2)
            for k in range(2):
                nc.tensor.matmul(ps[:], thw_sl(k),
                                 x_sb[k][:, tsl],
                                 start=(k == 0), stop=(k == 1))
            nc.vector.tensor_scalar_add(theta_sb[:, tsl], ps[:], tb_sb[:])
        ps = ps_pool.tile([P, 512], F32, name=f"ph_ps{t}", tag="pp",
                          bufs=2)
        for k in range(2):
            nc.tensor.matmul(ps[:], phw_sl(k),
                             x_sb[k][:, tsl],
                             start=(k == 0), stop=(k == 1))
        nc.vector.tensor_copy(phi_sb[:, tsl], ps[:])

    def gproj(t):
        # gT directly: x chunks stationary, g_w moving -> [j, IC] psum
        tsl = slice(t * 512, (t + 1) * 512)
        gt_ps = ps_pool.tile([P, 512], F32, name=f"gt_ps{t}", tag="pp",
                             bufs=2)
        for c in range(4):
            jc = 4 * t + c
            for k in range(2):
                nc.tensor.matmul(
                    gt_ps[:, c * P:(c + 1) * P],
                    x_sb[k][:, jc * P:(jc + 1) * P],
                    gw_sl(k),
                    start=(k == 0), stop=(k == 1))
        nc.vector.tensor_copy(gT_sb[:, tsl], gt_ps[:])

    proj(0)
    for gg in (0, 1):
        _emit_group(nc, pools, 0, gg)
    nc.vector.tensor_copy(wef_sb[:], wef_f[:])
    for t in range(1, 8):
        proj(t)
        gproj(t - 1)
        for gg in (2 * t, 2 * t + 1):
            _emit_group(nc, pools, 0, gg)
    gproj(7)

    # ---- block flushes; a block's AV flush interleaves with the NEXT
    # block's first groups so the exp stream never starves, and the tail
    # leads the DVE FIFO.
    def flush_and_transition(blk):
        nxt = blk + 1
        for grp in range(NGRP - DEFER, NGRP):
            _emit_consume(nc, pools, blk, grp, only="den")
        _emit_recip(nc, pools, blk)
        if nxt < NBLK:
            _emit_group(nc, pools, nxt, 0)
            _emit_consume(nc, pools, blk, NGRP - 4, only="av")
            _emit_group(nc, pools, nxt, 1)
            _emit_consume(nc, pools, blk, NGRP - 3, only="av")
            _emit_group(nc, pools, nxt, 2)
            _emit_consume(nc, pools, blk, NGRP - 2, only="av")
            _emit_consume(nc, pools, blk, NGRP - 1, only="av")
            _emit_attnmul(nc, pools, blk)
            _emit_group(nc, pools, nxt, 3)
            _emit_tail_pe(nc, pools, blk, yout)
        else:
            for grp in range(NGRP - DEFER, NGRP):
                _emit_consume(nc, pools, blk, grp, only="av")
            _emit_attnmul(nc, pools, blk)
            _emit_tail_pe(nc, pools, blk, yout, last=True)

    flush_and_transition(0)
    for blk in range(1, NBLK):
        for grp in range(4, NGRP):
            _emit_group(nc, pools, blk, grp)
        flush_and_transition(blk)


_CACHE = {}


def _build():
    if "nc" in _CACHE:
        return _CACHE["nc"]
    nc = bacc.Bacc("TRN2", target_bir_lowering=False, debug=False,
                   enable_asserts=False, num_devices=1)
    ins = {
        "xin": nc.dram_tensor("xin", [C, N], F16, kind="ExternalInput").ap(),
        "wcat": nc.dram_tensor("wcat", [C, 3 * IC], F16,
                               kind="ExternalInput").ap(),
        "wef": nc.dram_tensor("wef", [IC, C], F32, kind="ExternalInput").ap(),
        "tb": nc.dram_tensor("tb", [IC], F32, kind="ExternalInput").ap(),
        "bfin": nc.dram_tensor("bfin", [C], F32, kind="ExternalInput").ap(),
    }
    yout = nc.dram_tensor("yout", [C, HALF], F16, kind="ExternalOutput").ap()
    with tile.TileContext(nc) as tc:
        with ExitStack() as ctx:
            _kernel_body(ctx, tc, ins, yout)
    nc.compile()
    _CACHE["nc"] = nc
    return nc


def _host_prepare(inputs):
    """Host-side folds + per-core input maps."""
    ii = {k: np.ascontiguousarray(np.asarray(v, dtype=np.float32))
          for k, v in inputs.items()}
    inv = ii["bn_gamma"] / np.sqrt(ii["bn_var"] + BN_EPS)
    w_eff = ii["w_w"] * inv[:, None]                       # [C, IC]
    b_final = (w_eff @ ii["g_b"] + ii["w_b"] * inv
               + ii["bn_beta"] - ii["bn_mean"] * inv)      # [C]
    wcat = np.concatenate(
        [ii["theta_w"].T, ii["phi_w"].T, ii["g_w"].T], axis=1)  # [C, 3*IC]
    shared = {
        "wcat": np.ascontiguousarray(wcat).astype(np.float16),
        "wef": np.ascontiguousarray(w_eff.T),              # [IC, C]
        "tb": ii["theta_b"],
        "bfin": np.ascontiguousarray(b_final),
    }
    x = ii["x"].reshape(B, C, N)
    in_maps = []
    for core in range(NCORES):
        b, h = divmod(core, 2)
        own = x[b][:, h * HALF:(h + 1) * HALF]
        oth = x[b][:, (1 - h) * HALF:(2 - h) * HALF]
        xin = np.concatenate([own, oth], axis=1).astype(np.float16)
        in_maps.append({"xin": np.ascontiguousarray(xin), **shared})
    return in_maps


def _gather(results, x_dtype):
    out = np.empty((B, C, N), dtype=np.float32)
    for core in range(NCORES):
        b, h = divmod(core, 2)
        out[b][:, h * HALF:(h + 1) * HALF] = np.asarray(
            results[core]["yout"], dtype=np.float32)
    return out.reshape(B, C, H, W).astype(x_dtype, copy=False)


def kernel(**inputs):
    nc = _build()
    in_maps = _host_prepare(inputs)
    res = run_bass_kernel_spmd(nc, in_maps, core_ids=list(range(NCORES)))
    return _gather(res.results, np.asarray(inputs["x"]).dtype)


# revision 46
# speedup vs baseline: 1.0339x; 1.0339x over previous
# NonLocalBlock Trainium2 Bass kernel.
#
# Reference computation (per batch b):
#   theta = theta_w @ X + theta_b          [IC, N]   (X = x[b] as [C, N])
#   phi   = phi_w   @ X + phi_b            [IC, N]
#   g     = g_w     @ X + g_b              [IC, N]
#   attn  = softmax_j(theta^T phi)         [N, N]
#   att   = g @ attn^T                     [IC, N]
#   y     = BN(w_w @ att + w_b) + x
#
# Math folds used on device (validated vs reference):
#   - phi bias drops out of softmax entirely (adds an i-only constant).
#   - g bias folds into the final bias because attn rows sum to 1.
#   - BN is affine: fold into w_eff = inv*w_w and b_final.
#   - scores bounded (|s| < 50) so exp() needs no max-subtraction.
#
# Sharding: 8 cores = 4 batches x 2 row-halves. Each core receives x[b]
# with its own half's columns swapped to the front, so every core runs an
# identical program (pure SPMD): it projects theta for columns 0..2047
# ("own" rows i) and phi/g for all 4096 columns (keys/values j), computes
# 2048x4096 attention flash-style, and emits y for its own 2048 columns.
#
# Precision plan (validated vs reference in numpy, absmax-rel ~3.0e-3):
#   - x and the three projection weights ship as fp16 (halves the input
#     DMA, which floor-limits the startup) — fp16's 10 mantissa bits keep
#     the softmax scores accurate where bf16 would cost ~1.6e-2.
#   - theta/phi live in SBUF as fp32r; scores accumulate fp32 in PSUM.
#   - value path (exp output, gT, attn, w_eff) is bf16: attention-weighted
#     averaging damps value-path quantization.
#   - output ships fp16 (host upconverts); residual adds use fp16 x.
#
# Denominator: softmax denominators need a cross-partition sum, which only
# the PE (ones-matmul) can do cheaply.  exp groups are pairwise-summed on
# DVE (bf16, 2x mode) twice, so only 8+2 ones-matmuls per i-block reach
# the PE; the final pair skips the presum and feeds the PE directly so the
# block-flush critical path doesn't wait on the DVE queue.
#
# Schedule notes (from HW traces):
#   - a short burst of 128-col bf16 warmup matmuls at t=0 starts the PE
#     HAM clock ramp; a dummy exp preloads the ACT exp table.
#   - x streams over the sync (C-rows 0:128) and gpsimd (128:256) DMA
#     rings; ALL weights go on the scalar ring so no weight transfer ever
#     delays an x slice.  gT is produced directly by x-chunk-stationary
#     matmuls (no PE transposes, no second copy).
#   - per 512-col slice t: projections, then attention groups (2t, 2t+1)
#     of block 0 — the exp stream starts ~10us in and paces the kernel.
#   - exp consumption (AV matmuls + presums) runs DEFER groups late; block
#     tails (reciprocal/normalize/W/store) interleave with the next
#     block's first groups so no engine FIFO ever stalls on them.

from contextlib import ExitStack

import numpy as np

import concourse.bass as bass
import concourse.tile as tile
from concourse import bacc, mybir
from concourse.bass_utils import run_bass_kernel_spmd

F32 = mybir.dt.float32
F32R = mybir.dt.float32r
F16 = mybir.dt.float16
BF16 = mybir.dt.bfloat16
AF = mybir.ActivationFunctionType
ALU = mybir.AluOpType

B, C, IC = 4, 256, 128
H = W = 64
N = H * W            # 4096
HALF = N // 2        # 2048 rows of attention per core
P = 128
NCORES = 8
NBLK = HALF // 512   # 4 i-blocks of 512
NCH = N // P         # 32 j-chunks of 128
NGRP = NCH // 2      # 16 groups of 2 chunks per i-block
DEFER = 4            # consume exp output this many groups late
NWARM = 48           # HAM warmup matmuls at t=0 (128-col bf16, ~5.1us)
# Groups whose exp runs on DVE via bf16 Schraudolph (bit-trick exp: bf16
# bits of e^s are ~ s*128/ln2 + 127*128 - C) instead of the saturated ACT
# engine.  3 of 16 groups rebalances ACT 18.4->14.9us/block while DVE
# stays under the PE pace.  Costs ~5.5e-3 absmax-rel (validated, C=5).
SCHRAUD_GRPS = (4, 8, 12)
SCH_A = 128.0 / float(np.log(2.0))
SCH_B = 127.0 * 128.0 - 5.0
BN_EPS = 1e-5


def _r(ap):
    return ap.bitcast(F32R)


def _emit_consume(nc, pools, blk, grp, only=None):
    """AV matmuls + denominator work for group `grp`.

    Denominator: groups 0..13 run a bf16 presum ladder on DVE (pairs of
    chunks, then pairs of groups) feeding one ones-matmul per group-pair;
    groups 14/15 feed the ones-matmul directly so the block flush never
    waits on the DVE queue."""
    att_ps = pools["att_ps"][blk]
    den_ps = pools["den_ps"][blk]
    gT_sb, onesP_sb = pools["gT_sb"], pools["onesP_sb"]
    p_pool = pools["pre"]
    ex_sb = pools["ex_sbs"][(blk, grp)]
    if only in (None, "den"):
        # every block's final pair skips the presum ladder: at flush time
        # the DVE queue is busy with tail ops, and den matmuls waiting on
        # flush-time presums would stall the PE at each block boundary.
        if grp < NGRP - 2:
            p1 = p_pool.tile([P, 512], BF16, name=f"p1_{blk}_{grp}",
                             tag="p1", bufs=3)
            nc.vector.tensor_add(p1[:], ex_sb[:, 0:512], ex_sb[:, 512:1024])
            pools["p1_sbs"][(blk, grp)] = p1
            if grp % 2 == 1:
                pa = pools["p1_sbs"].pop((blk, grp - 1))
                p2 = p_pool.tile([P, 512], BF16, name=f"p2_{blk}_{grp // 2}",
                                 tag="p2", bufs=3)
                nc.vector.tensor_add(p2[:], pa[:], p1[:])
                nc.tensor.matmul(den_ps[:], onesP_sb[:], p2[:],
                                 start=grp == 1, stop=False)
        else:
            for c in range(2):
                jc = grp * 2 + c
                nc.tensor.matmul(
                    den_ps[:], onesP_sb[:],
                    ex_sb[:, c * 512:(c + 1) * 512],
                    start=False, stop=jc == NCH - 1)
    if only in (None, "av"):
        for c in range(2):
            jc = grp * 2 + c
            nc.tensor.matmul(
                att_ps[:], gT_sb[:, jc * P:(jc + 1) * P],
                ex_sb[:, c * 512:(c + 1) * 512],
                start=jc == 0, stop=jc == NCH - 1)


def _emit_group(nc, pools, blk, grp):
    """Scores + exp for one [128,1024] group, consuming DEFER groups late."""
    ps_pool, ex_pool = pools["ps"], pools["ex"]
    theta_sb, phi_sb = pools["theta_sb"], pools["phi_sb"]
    isl = slice(blk * 512, (blk + 1) * 512)
    if grp == 0:
        pools["att_ps"][blk] = ps_pool.tile(
            [P, 512], F32, name=f"att_ps{blk}", tag="att", bufs=1)
        pools["den_ps"][blk] = ps_pool.tile(
            [P, 512], F32, name=f"den_ps{blk}", tag="den", bufs=1)
    sc_ps = ps_pool.tile([P, 1024], F32, name=f"sc{blk}_{grp}", tag="sc",
                         bufs=2)
    for c in range(2):
        jc = grp * 2 + c
        nc.tensor.matmul(
            sc_ps[:, c * 512:(c + 1) * 512],
            phi_sb[:, jc * P:(jc + 1) * P],
            theta_sb[:, isl],
            start=True, stop=True)
    ex_sb = ex_pool.tile([P, 1024], BF16, name=f"ex{blk}_{grp}", tag="ex")
    pools["ex_sbs"][(blk, grp)] = ex_sb
    if grp in SCHRAUD_GRPS:
        nc.vector.tensor_scalar(
            out=ex_sb[:].bitcast(mybir.dt.int16), in0=sc_ps[:],
            scalar1=SCH_A, scalar2=SCH_B, op0=ALU.mult, op1=ALU.add)
    else:
        nc.scalar.activation(ex_sb[:], sc_ps[:], AF.Exp)
    if grp >= DEFER:
        _emit_consume(nc, pools, blk, grp - DEFER)


def _emit_recip(nc, pools, blk):
    """1/den — emitted right after the block's den flush so it frees the
    den PSUM slot early and leads the DVE FIFO."""
    rec_pool = pools["rec"]
    den_ps = pools["den_ps"][blk]
    recb = rec_pool.tile([P, 512], F32, name=f"recb{blk}", tag="recb")
    nc.vector.reciprocal_approx_fast(out=recb[:], in_=den_ps[:])
    pools["recbs"][blk] = recb


def _emit_attnmul(nc, pools, blk):
    """attn = att * (1/den) — must follow the block's last AV matmul."""
    rec_pool = pools["rec"]
    att_ps = pools["att_ps"][blk]
    recb = pools["recbs"].pop(blk)
    attn_sb = rec_pool.tile([P, 512], BF16, name=f"attn{blk}", tag="attn")
    nc.vector.tensor_mul(attn_sb[:], att_ps[:], recb[:])
    pools["attn_sbs"][blk] = attn_sb


def _emit_tail_pe(nc, pools, blk, yout, last=False):
    """W projection, bias+residual, fp16 store for block `blk`."""
    ps_pool, rec_pool = pools["ps"], pools["rec"]
    wef_sb, bfin_sb = pools["wef_sb"], pools["bfin_sb"]
    x_sb = pools["x_sb"]
    attn_sb = pools["attn_sbs"].pop(blk)
    isl = slice(blk * 512, (blk + 1) * 512)
    for k in range(2):
        y_ps = ps_pool.tile([P, 512], F32, name=f"y{blk}_{k}", tag="pp",
                            bufs=2)
        nc.tensor.matmul(
            y_ps[:], wef_sb[:, k * P:(k + 1) * P], attn_sb[:],
            start=True, stop=True)
        yo = rec_pool.tile([P, 512], F16, name=f"yo{blk}_{k}", tag="yo")
        # yo = (y + b_final) + x  in one DVE op
        nc.vector.scalar_tensor_tensor(
            out=yo[:], in0=y_ps[:], scalar=bfin_sb[:, k:k + 1],
            in1=x_sb[k][:, isl], op0=ALU.add, op1=ALU.add)
        if last:
            eng = nc.sync if k == 0 else nc.scalar
        else:
            eng = nc.sync if k == 0 else nc.gpsimd
        eng.dma_start(out=yout[k * P:(k + 1) * P, isl], in_=yo[:])


def _kernel_body(ctx, tc, ins, yout):
    nc = tc.nc
    xin, wef, tb, bfin = ins["xin"], ins["wef"], ins["tb"], ins["bfin"]

    consts = ctx.enter_context(tc.tile_pool(name="consts", bufs=1))
    big = ctx.enter_context(tc.tile_pool(name="big", bufs=1))

    # ---- dummy tiles for HAM warmup (bf16 128-col matmuls) ----
    dum_f = consts.tile([P, P], F32, name="dum_f")
    nc.vector.memset(dum_f[:], 1.0)
    dum_b = consts.tile([P, P], BF16, name="dum_b")
    nc.vector.tensor_copy(dum_b[:], dum_f[:])

    # ---- x load: k=0 C-half on the sync ring, k=1 on the gpsimd ring;
    # all weights on the scalar ring (idle until the exps start).
    x_sb = [big.tile([P, N], F16, name=f"x_sb{k}") for k in range(2)]

    def xdma(t, k, eng):
        tsl = slice(t * 512, (t + 1) * 512)
        eng.dma_start(out=x_sb[k][:, tsl],
                      in_=xin[k * P:(k + 1) * P, tsl])

    wcat_sb = consts.tile([P, 2 * 384], F16, name="wcat_sb")
    tb_sb = consts.tile([P, 1], F32, name="tb_sb")
    wef_f = consts.tile([P, C], F32, name="wef_f")
    bfin_sb = consts.tile([P, 2], F32, name="bfin_sb")
    wcat = ins["wcat"]

    def xchunk(c0, c1, k, eng):
        eng.dma_start(out=x_sb[k][:, c0:c1],
                      in_=xin[k * P:(k + 1) * P, c0:c1])

    # sync and scalar are hardware-dynamic DMA queues; the gpsimd queue is
    # software-dynamic (~2x slower), so it only gets the tail k=1 chunk
    # (~8us of slack).  A queue's first couple of descriptors cost ~2us
    # each before transfers pipeline, so thw|phw|gw ship as ONE
    # concatenated [C, 384] array (wcat) in two descriptors at the head of
    # the fast sync queue, with x right behind in first-use order.
    nc.sync.dma_start(out=wcat_sb[:].rearrange("p (k m) -> p k m", k=2),
                      in_=wcat.rearrange("(k p) m -> p k m", p=P))
    nc.scalar.dma_start(out=tb_sb[:], in_=tb[:, None])
    exdum = consts.tile([P, 1], F32, name="exdum")
    nc.scalar.activation(exdum[:], dum_f[:, 0:1], AF.Exp)  # load exp table
    xchunk(0, 512, 0, nc.sync)
    xchunk(0, 512, 1, nc.sync)
    xchunk(512, 1024, 0, nc.sync)
    xchunk(512, 1024, 1, nc.sync)
    xchunk(1024, 2048, 0, nc.sync)
    xchunk(2048, 3072, 0, nc.sync)
    xchunk(3072, 4096, 0, nc.sync)
    xchunk(1024, 2048, 1, nc.scalar)
    xchunk(2048, 4096, 1, nc.gpsimd)
    nc.scalar.dma_start(out=wef_f[:], in_=wef[:, :])
    nc.scalar.dma_start(out=bfin_sb[:],
                        in_=bfin.rearrange("(k p) -> p k", p=P))

    def thw_sl(k):
        return wcat_sb[:, k * 384:k * 384 + P]

    def phw_sl(k):
        return wcat_sb[:, k * 384 + P:k * 384 + 2 * P]

    def gw_sl(k):
        return wcat_sb[:, k * 384 + 2 * P:k * 384 + 3 * P]

    wef_sb = consts.tile([P, C], BF16, name="wef_sb")
    onesP_sb = consts.tile([P, P], BF16, name="onesP_sb")
    nc.vector.memset(onesP_sb[:], 1.0)

    theta_sb = big.tile([P, HALF], F32R, name="theta_sb")
    phi_sb = big.tile([P, N], F32R, name="phi_sb")
    gT_sb = big.tile([P, N], BF16, name="gT_sb")

    # ---- single PSUM pool, tagged slots (8 banks total):
    #   sc 2x[128,1024]=4, att 1, den 1, pp 2x[128,512]=2 (proj/gT/y)
    ps_pool = ctx.enter_context(tc.tile_pool(name="ps", bufs=1, space="PSUM"))
    pools = {
        "ps": ps_pool,
        "ex": ctx.enter_context(tc.tile_pool(name="ex", bufs=5 + DEFER)),
        "pre": ctx.enter_context(tc.tile_pool(name="pre", bufs=1)),
        "rec": ctx.enter_context(tc.tile_pool(name="rec", bufs=2)),
        "theta_sb": theta_sb, "phi_sb": phi_sb, "gT_sb": gT_sb,
        "onesP_sb": onesP_sb, "wef_sb": wef_sb, "bfin_sb": bfin_sb,
        "x_sb": x_sb,
        "att_ps": {}, "den_ps": {}, "ex_sbs": {},
        "p1_sbs": {}, "attn_sbs": {}, "recbs": {},
    }

    # ---- phase 1: per-slice projections + block-0 groups as x lands ----
    dum_ps = ps_pool.tile([P, 512], F32, name="dum_ps", tag="pp", bufs=2)
    for i in range(NWARM):
        nc.tensor.matmul(dum_ps[:, 0:P], dum_b[:], dum_b[:],
                         start=True, stop=True)

    def proj(t):
        tsl = slice(t * 512, (t + 1) * 512)
        if t < NBLK:
            ps = ps_pool.tile([P, 512], F32, name=f"th_ps{t}", tag="pp",
                              bufs=2)
            for k in range(2):
                nc.tensor.matmul(ps[:], thw_sl(k),
                                 x_sb[k][:, tsl],
                                 start=(k == 0), stop=(k == 1))
            nc.vector.tensor_scalar_add(theta_sb[:, tsl], ps[:], tb_sb[:])
        ps = ps_pool.tile([P, 512], F32, name=f"ph_ps{t}", tag="pp",
                          bufs=2)
        for k in range(2):
            nc.tensor.matmul(ps[:], phw_sl(k),
                             x_sb[k][:, tsl],
                             start=(k == 0), stop=(k == 1))
        nc.vector.tensor_copy(phi_sb[:, tsl], ps[:])

    def gproj(t):
        # gT directly: x chunks stationary, g_w moving -> [j, IC] psum
        tsl = slice(t * 512, (t + 1) * 512)
        gt_ps = ps_pool.tile([P, 512], F32, name=f"gt_ps{t}", tag="pp",
                             bufs=2)
        for c in range(4):
            jc = 4 * t + c
            for k in range(2):
                nc.tensor.matmul(
                    gt_ps[:, c * P:(c + 1) * P],
                    x_sb[k][:, jc * P:(jc + 1) * P],
                    gw_sl(k),
                    start=(k == 0), stop=(k == 1))
        nc.vector.tensor_copy(gT_sb[:, tsl], gt_ps[:])

    proj(0)
    for gg in (0, 1):
        _emit_group(nc, pools, 0, gg)
    nc.vector.tensor_copy(wef_sb[:], wef_f[:])
    for t in range(1, 8):
        proj(t)
        gproj(t - 1)
        for gg in (2 * t, 2 * t + 1):
            _emit_group(nc, pools, 0, gg)
    gproj(7)

    # ---- block flushes; a block's AV flush interleaves with the NEXT
    # block's first groups so the exp stream never starves, and the tail
    # leads the DVE FIFO.
    def flush_and_transition(blk):
        nxt = blk + 1
        for grp in range(NGRP - DEFER, NGRP):
            _emit_consume(nc, pools, blk, grp, only="den")
        _emit_recip(nc, pools, blk)
        if nxt < NBLK:
            _emit_group(nc, pools, nxt, 0)
            _emit_consume(nc, pools, blk, NGRP - 4, only="av")
            _emit_group(nc, pools, nxt, 1)
            _emit_consume(nc, pools, blk, NGRP - 3, only="av")
            _emit_group(nc, pools, nxt, 2)
            _emit_consume(nc, pools, blk, NGRP - 2, only="av")
            _emit_consume(nc, pools, blk, NGRP - 1, only="av")
            _emit_attnmul(nc, pools, blk)
            _emit_group(nc, pools, nxt, 3)
            _emit_tail_pe(nc, pools, blk, yout)
        else:
            for grp in range(NGRP - DEFER, NGRP):
                _emit_consume(nc, pools, blk, grp, only="av")
            _emit_attnmul(nc, pools, blk)
            _emit_tail_pe(nc, pools, blk, yout, last=True)

    flush_and_transition(0)
    for blk in range(1, NBLK):
        for grp in range(4, NGRP):
            _emit_group(nc, pools, blk, grp)
        flush_and_transition(blk)


_CACHE = {}


def _build():
    if "nc" in _CACHE:
        return _CACHE["nc"]
    nc = bacc.Bacc("TRN2", target_bir_lowering=False, debug=False,
                   enable_asserts=False, num_devices=1)
    ins = {
        "xin": nc.dram_tensor("xin", [C, N], F16, kind="ExternalInput").ap(),
        "wcat": nc.dram_tensor("wcat", [C, 3 * IC], F16,
                               kind="ExternalInput").ap(),
        "wef": nc.dram_tensor("wef", [IC, C], F32, kind="ExternalInput").ap(),
        "tb": nc.dram_tensor("tb", [IC], F32, kind="ExternalInput").ap(),
        "bfin": nc.dram_tensor("bfin", [C], F32, kind="ExternalInput").ap(),
    }
    yout = nc.dram_tensor("yout", [C, HALF], F16, kind="ExternalOutput").ap()
    with tile.TileContext(nc) as tc:
        with ExitStack() as ctx:
            _kernel_body(ctx, tc, ins, yout)
    nc.compile()
    _CACHE["nc"] = nc
    return nc


def _host_prepare(inputs):
    """Host-side folds + per-core input maps."""
    ii = {k: np.ascontiguousarray(np.asarray(v, dtype=np.float32))
          for k, v in inputs.items()}
    inv = ii["bn_gamma"] / np.sqrt(ii["bn_var"] + BN_EPS)
    w_eff = ii["w_w"] * inv[:, None]                       # [C, IC]
    b_final = (w_eff @ ii["g_b"] + ii["w_b"] * inv
               + ii["bn_beta"] - ii["bn_mean"] * inv)      # [C]
    wcat = np.concatenate(
        [ii["theta_w"].T, ii["phi_w"].T, ii["g_w"].T], axis=1)  # [C, 3*IC]
    shared = {
        "wcat": np.ascontiguousarray(wcat).astype(np.float16),
        "wef": np.ascontiguousarray(w_eff.T),              # [IC, C]
        "tb": ii["theta_b"],
        "bfin": np.ascontiguousarray(b_final),
    }
    x = ii["x"].reshape(B, C, N)
    in_maps = []
    for core in range(NCORES):
        b, h = divmod(core, 2)
        own = x[b][:, h * HALF:(h + 1) * HALF]
        oth = x[b][:, (1 - h) * HALF:(2 - h) * HALF]
        xin = np.concatenate([own, oth], axis=1).astype(np.float16)
        in_maps.append({"xin": np.ascontiguousarray(xin), **shared})
    return in_maps


def _gather(results, x_dtype):
    out = np.empty((B, C, N), dtype=np.float32)
    for core in range(NCORES):
        b, h = divmod(core, 2)
        out[b][:, h * HALF:(h + 1) * HALF] = np.asarray(
            results[core]["yout"], dtype=np.float32)
    return out.reshape(B, C, H, W).astype(x_dtype, copy=False)


def kernel(**inputs):
    nc = _build()
    in_maps = _host_prepare(inputs)
    res = run_bass_kernel_spmd(nc, in_maps, core_ids=list(range(NCORES)))
    return _gather(res.results, np.asarray(inputs["x"]).dtype)
